# revision 1
# baseline (speedup 1.0000x reference)
"""Trainium2 Bass kernel for 3-layer relational GCN message passing.

Math (per layer, from the reference):
  deg[c]  = #edges with col==c          (over ALL edges, fixed across layers)
  dis     = where(deg>0, deg^-0.5, 0)
  out[r]  = leakyrelu( dis[r] * sum_t (sum_{e: row=r,type=t} w_e * (dis*x)[col_e]) @ W_t + b )
final = (x + z1 + z2 + z3) / 4

Strategy (8 NeuronCores):
  - Node-ownership sharding: core k owns rows [6250k, 6250(k+1)); every edge is
    assigned to the core owning its destination row.
  - Per layer, each core gathers source-node vectors for its edges from a
    replicated bf16 table in DRAM (dma_gather, edges land on partitions),
    scatter-reduces them into a feat-major accumulator A via PE matmuls with
    host-built edge-weight-valued one-hot "S" tiles (fp32 PSUM accumulate),
    then a 4-type GEMM (A-chunk as stationary) produces node-major output.
  - Per-node scalars (dis) are applied with ACT per-partition scale; bias is a
    host-replicated [128,128] tile added on DVE; leaky-relu on ACT (Lrelu).
  - Cores exchange layer outputs with an AllGather collective (bf16 table).
"""

import os
import sys
import numpy as np

sys.path.insert(0, "/opt/trn_rl_repo")

N = 50000
D = 128
NT = 4
NCORES = 8
RPC_REAL = 6250          # real rows per core
RPC = 6272               # padded rows per core (49*128)
NCHUNK = 49              # row chunks of 128 per core
NPAD = NCORES * RPC      # padded table rows = 50176
HALF = 25088             # int16 split point for gather indices (= 4*RPC)
NDEST = NT * RPC         # destination slots per core = 25088
WIN = 64                 # dests per PSUM window
NW = NDEST // WIN        # 392 windows
CHUNKS = 14
WPC = NW // CHUNKS       # 28 windows per chunk
NEG_SLOPE = 0.01

_CACHE = {}


def _build_host_data(edge_index, edge_type, edge_attr):
    """Integer-only edge bookkeeping + dense S-tile layout (input formatting)."""
    row = edge_index[0].astype(np.int64)
    col = edge_index[1].astype(np.int64)
    et = edge_type.astype(np.int64)
    w = edge_attr.astype(np.float32)

    deg = np.bincount(col, minlength=N).astype(np.float32)

    owner = row // RPC_REAL
    r_loc = row % RPC_REAL
    dest = et * RPC + r_loc                       # [0, NDEST)
    tcol = (col // RPC_REAL) * RPC + (col % RPC_REAL)  # padded table row
    stream = (tcol >= HALF).astype(np.int64)
    winid = dest // WIN

    # Per-core, per-window, per-stream edge lists (sorted bucketing).
    order = np.lexsort((stream, winid, owner))
    row_s, dest_s, tcol_s, w_s = row[order], dest[order], tcol[order], w[order]
    owner_s, win_s, str_s = owner[order], winid[order], stream[order]

    # counts[k, w, s]
    key = (owner_s * NW + win_s) * 2 + str_s
    cnt = np.bincount(key, minlength=NCORES * NW * 2).reshape(NCORES, NW, 2)
    starts = np.zeros(NCORES * NW * 2 + 1, dtype=np.int64)
    np.cumsum(cnt.reshape(-1), out=starts[1:])

    # Global uniform tile profile: T[w, s] tiles of 128 slots.
    T = np.ceil(cnt.max(axis=0) / 128).astype(np.int64)  # [NW, 2]
    empty = T.sum(axis=1) == 0
    T[empty, 0] = 1

    ntiles = int(T.sum())
    # per-chunk gather sizes (slots) per stream
    Twc = T.reshape(CHUNKS, WPC, 2)
    g_slots = Twc.sum(axis=1) * 128               # [CHUNKS, 2]
    g_off = np.zeros((CHUNKS, 2), dtype=np.int64)  # slot offset within stream buf
    # idx tensor column offsets (units of 16 idxs) per (chunk, stream), in
    # concatenated (chunk-major, stream-minor) order
    idx_off = np.zeros((CHUNKS, 2), dtype=np.int64)
    acc_idx = 0
    for c in range(CHUNKS):
        for s in range(2):
            g_off[c, s] = 0  # per-chunk buffers start at 0
            idx_off[c, s] = acc_idx
            acc_idx += g_slots[c, s] // 16
    tot_idx_cols = acc_idx

    # Static schedule: for each chunk, for each window, list of
    # (stream, block_index_in_chunk_gather_buffer, global_tile_index)
    sched = []
    gtile = 0
    for c in range(CHUNKS):
        blk = [0, 0]
        cw = []
        for wl in range(WPC):
            wg = c * WPC + wl
            tl = []
            for s in range(2):
                for _ in range(int(T[wg, s])):
                    tl.append((s, blk[s], gtile))
                    blk[s] += 1
                    gtile += 1
            cw.append(tl)
        sched.append(cw)
    assert gtile == ntiles

    # Per-core data arrays.
    per_core = []
    for k in range(NCORES):
        idx16 = np.zeros((tot_idx_cols, 16), dtype=np.int16)  # [cols, 16] -> T
        S = np.zeros((ntiles * WIN, 128), dtype=np.float32)   # [tile*WIN, slot]
        gtile_k = 0
        for c in range(CHUNKS):
            for s in range(2):
                base = idx_off[c, s]
                nslots = int(g_slots[c, s])
                vals = np.zeros(nslots, dtype=np.int16)
                # fill below per window
                # store later
                idx16_flat = vals
                # iterate windows of this chunk for stream s
                slot_pos = 0
                for wl in range(WPC):
                    wg = c * WPC + wl
                    st = starts[(k * NW + wg) * 2 + s]
                    en = starts[(k * NW + wg) * 2 + s + 1]
                    ne = int(en - st)
                    ntw = int(T[wg, s])
                    take = min(ne, ntw * 128)
                    assert ne <= ntw * 128
                    idx16_flat[slot_pos:slot_pos + take] = (
                        tcol_s[st:st + take] - s * HALF).astype(np.int16)
                    slot_pos += ntw * 128
                assert slot_pos == nslots
                idx16[base:base + nslots // 16] = idx16_flat.reshape(-1, 16)
            # S entries for this chunk's tiles (both streams, window-ordered)
            for wl in range(WPC):
                wg = c * WPC + wl
                for (s, blk, gt) in sched[c][wl]:
                    # which 128-slot range of (k, wg, s) does this tile cover?
                    st = starts[(k * NW + wg) * 2 + s]
                    en = starts[(k * NW + wg) * 2 + s + 1]
                    # block index within the window for this stream:
                    # count how many earlier tiles of same (wg, s)
                    prev = sum(1 for (s2, b2, g2) in sched[c][wl]
                               if s2 == s and g2 < gt)
                    lo = st + prev * 128
                    hi = min(en, lo + 128)
                    if hi > lo:
                        sl = np.arange(lo, hi)
                        pslot = (sl - lo).astype(np.int64)
                        dcol = (dest_s[sl] - wg * WIN).astype(np.int64)
                        S[gt * WIN + dcol, pslot] = w_s[sl]
        # S laid out [128 part, ntiles*WIN] bf16 on device; host shape
        # [ntiles*WIN, 128] -> transpose at upload
        per_core.append({"idx16": idx16, "S": S})

    # per-core x / deg blocks
    meta = {
        "T": T, "sched": sched, "g_slots": g_slots, "idx_off": idx_off,
        "tot_idx_cols": tot_idx_cols, "ntiles": ntiles, "deg": deg,
    }
    return per_core, meta


def _build_program(meta):
    from concourse import bass, bacc, mybir, tile

    dt = mybir.dt
    Afn = mybir.ActivationFunctionType
    Alu = mybir.AluOpType

    g_slots = meta["g_slots"]
    idx_off = meta["idx_off"]
    sched = meta["sched"]
    tot_idx_cols = meta["tot_idx_cols"]
    ntiles = meta["ntiles"]
    maxblk = [int(max(g_slots[c, s] // 128 for c in range(CHUNKS))) for s in (0, 1)]
    tiles_per_chunk = [sum(len(tl) for tl in sched[c]) for c in range(CHUNKS)]
    max_tpc = max(tiles_per_chunk)

    nc = bacc.Bacc("TRN2", target_bir_lowering=False, debug=False,
                   num_devices=NCORES)

    # External I/O
    x_in = nc.dram_tensor("x_in", [RPC, D], dt.float32, kind="ExternalInput").ap()
    deg_in = nc.dram_tensor("deg_in", [128, NCHUNK], dt.float32,
                            kind="ExternalInput").ap()
    idx_in = nc.dram_tensor("idx_in", [128, tot_idx_cols], dt.int16,
                            kind="ExternalInput").ap()
    s_in = nc.dram_tensor("s_in", [128, ntiles * WIN], dt.bfloat16,
                          kind="ExternalInput").ap()
    w_in = nc.dram_tensor("w_in", [3, NT, D, D], dt.float32,
                          kind="ExternalInput").ap()
    b_in = nc.dram_tensor("b_in", [3, 128, D], dt.float32,
                          kind="ExternalInput").ap()
    out_d = nc.dram_tensor("out_d", [RPC, D], dt.float32,
                           kind="ExternalOutput").ap()

    rg = [list(range(NCORES))]

    with tile.TileContext(nc) as tc:
        with tc.tile_pool(name="sb", bufs=1) as sb, \
             tc.tile_pool(name="sbA", bufs=1) as sbA, \
             tc.tile_pool(name="gpool", bufs=2) as gpool, \
             tc.tile_pool(name="spool", bufs=2) as spool, \
             tc.tile_pool(name="stage", bufs=3) as stage, \
             tc.tile_pool(name="psw", bufs=6, space="PSUM") as psw, \
             tc.tile_pool(name="psg", bufs=2, space="PSUM") as psg, \
             tc.tile_pool(name="dram", bufs=1, space="DRAM") as dram:

            # ---- persistent SBUF ----
            idx_sb = sb.tile([128, tot_idx_cols], dt.int16)
            nc.sync.dma_start(idx_sb[:], idx_in[:])
            dis = sb.tile([128, NCHUNK], dt.float32)
            degt = sb.tile([128, NCHUNK], dt.float32)
            nc.sync.dma_start(degt[:], deg_in[:])
            W_sb = sb.tile([128, 3, NT, D], dt.bfloat16)   # [f, l, t, o]
            for l in range(3):
                for t in range(NT):
                    wtmp = stage.tile([128, D], dt.float32, tag="wtmp")
                    nc.sync.dma_start(wtmp[:], w_in[l, t, :, :])
                    nc.vector.tensor_copy(W_sb[:, l, t, :], wtmp[:])
            B_sb = sb.tile([128, 3, D], dt.float32)
            for l in range(3):
                btmp = stage.tile([128, D], dt.float32, tag="wtmp")
                nc.sync.dma_start(btmp[:], b_in[l, :, :])
                nc.vector.tensor_copy(B_sb[:, l, :], btmp[:])
            acc = sb.tile([128, NCHUNK, D], dt.float32)

            # dis = where(deg>0, 1/sqrt(deg), 0)
            deg1 = stage.tile([128, NCHUNK], dt.float32, tag="d1")
            nc.vector.tensor_scalar_max(deg1[:], degt[:], 1.0)
            inv = stage.tile([128, NCHUNK], dt.float32, tag="d2")
            nc.vector.reciprocal(inv[:], deg1[:])
            sq = stage.tile([128, NCHUNK], dt.float32, tag="d3")
            nc.scalar.activation(sq[:], inv[:], Afn.Sqrt)
            mask = stage.tile([128, NCHUNK], dt.float32, tag="d4")
            nc.vector.tensor_scalar(mask[:], degt[:], 0.0, None,
                                    op0=Alu.is_gt)
            nc.vector.tensor_tensor(dis[:], sq[:], mask[:], op=Alu.mult)

            # ---- DRAM ping-pong tables and per-layer blocks ----
            REPS = int(os.environ.get("KREPS", "1"))
            tables = [dram.tile([NPAD, D], dt.bfloat16, name=f"tab{i}",
                                addr_space="Shared")
                      for i in range(3 * REPS)]
            blks = [dram.tile([RPC, D], dt.bfloat16, name=f"blk{i}")
                    for i in range(2)]

            # ---- prologue: acc = x ; table0 = AllGather(dis * x) ----
            for rc in range(NCHUNK):
                xc = stage.tile([128, D], dt.float32, tag="xc")
                nc.sync.dma_start(xc[:], x_in[rc * 128:(rc + 1) * 128, :])
                nc.vector.tensor_copy(acc[:, rc, :], xc[:])
                zt = stage.tile([128, D], dt.bfloat16, tag="zt")
                nc.scalar.activation(zt[:], xc[:], Afn.Copy,
                                     scale=dis[:, rc:rc + 1])
                nc.sync.dma_start(blks[0][rc * 128:(rc + 1) * 128, :], zt[:])
            nc.gpsimd.collective_compute(
                "AllGather", Alu.bypass, replica_groups=rg,
                ins=[blks[0].opt()], outs=[tables[0].opt()])

            # ---- layers ----
            PROBE = os.environ.get("KPROBE", "full")
            NLAYERS = 0 if PROBE == "a" else 3 * REPS
            for l in range(NLAYERS):
                lw = l % 3
                table = tables[l]
                A = sbA.tile([128, NDEST], dt.bfloat16, tag="A")
                for c in range(CHUNKS):
                    gts = []
                    for s in range(2):
                        nb = int(g_slots[c, s]) // 128
                        gt = gpool.tile([128, maxblk[s], D], dt.bfloat16,
                                        tag=f"g{s}")
                        src = table[s * HALF:(s + 1) * HALF, :] if s == 0 else \
                            table[HALF:NPAD, :]
                        io = int(idx_off[c, s])
                        nidx = int(g_slots[c, s])
                        if PROBE == "b0":
                            gts.append(gt)
                            continue
                        nc.gpsimd.dma_gather(
                            out_ap=gt[:, :nb, :],
                            in_ap=src,
                            idxs_ap=idx_sb[:, io:io + nidx // 16],
                            num_idxs=nidx,
                            num_idxs_reg=nidx,
                            elem_size=D,
                            single_packet=False,
                        )
                        gts.append(gt)
                    # S for this chunk
                    tpc = tiles_per_chunk[c]
                    t0 = sum(tiles_per_chunk[:c])
                    s_sb = spool.tile([128, max_tpc * WIN], dt.bfloat16,
                                      tag="S")
                    nc.sync.dma_start(s_sb[:, :tpc * WIN],
                                      s_in[:, t0 * WIN:(t0 + tpc) * WIN])
                    if PROBE in ("b", "b0"):
                        continue
                    for wl in range(WPC):
                        wg = c * WPC + wl
                        tl = sched[c][wl]
                        pw = psw.tile([128, WIN], dt.float32, tag="pw")
                        for i, (s, blk, gtile) in enumerate(tl):
                            soff = (gtile - t0) * WIN
                            nc.tensor.matmul(
                                out=pw[:],
                                lhsT=gts[s][:, blk, :],
                                rhs=s_sb[:, soff:soff + WIN],
                                start=(i == 0),
                                stop=(i == len(tl) - 1),
                            )
                        nc.scalar.activation(A[:, wg * WIN:(wg + 1) * WIN],
                                             pw[:], Afn.Copy)
                # GEMM + output per row chunk
                for rc in range(NCHUNK):
                    pg = psg.tile([128, D], dt.float32, tag="pg")
                    for t in range(NT):
                        nc.tensor.matmul(
                            out=pg[:],
                            lhsT=A[:, t * RPC + rc * 128: t * RPC + (rc + 1) * 128],
                            rhs=W_sb[:, lw, t, :],
                            start=(t == 0),
                            stop=(t == NT - 1),
                        )
                    tz = stage.tile([128, D], dt.float32, tag="tz")
                    nc.scalar.activation(tz[:], pg[:], Afn.Copy,
                                         scale=dis[:, rc:rc + 1])
                    nc.vector.tensor_tensor(tz[:], tz[:], B_sb[:, lw, :],
                                            op=Alu.add)
                    z = stage.tile([128, D], dt.float32, tag="z")
                    nc.scalar.activation(z[:], tz[:], Afn.Lrelu,
                                         alpha=NEG_SLOPE)
                    nc.vector.tensor_tensor(acc[:, rc, :], acc[:, rc, :], z[:],
                                            op=Alu.add)
                    if l < NLAYERS - 1:
                        zt = stage.tile([128, D], dt.bfloat16, tag="zt2")
                        nc.scalar.activation(zt[:], z[:], Afn.Copy,
                                             scale=dis[:, rc:rc + 1])
                        nc.sync.dma_start(
                            blks[(l + 1) % 2][rc * 128:(rc + 1) * 128, :],
                            zt[:])
                if l < NLAYERS - 1:
                    nc.gpsimd.collective_compute(
                        "AllGather", Alu.bypass, replica_groups=rg,
                        ins=[blks[(l + 1) % 2].opt()],
                        outs=[tables[l + 1].opt()])

            # ---- epilogue: out = acc / 4 ----
            for rc in range(NCHUNK):
                oc = stage.tile([128, D], dt.float32, tag="oc")
                nc.vector.tensor_scalar_mul(oc[:], acc[:, rc, :], 0.25)
                nc.sync.dma_start(out_d[rc * 128:(rc + 1) * 128, :], oc[:])

    nc.compile()
    return nc


def kernel(x, edge_index, edge_type, edge_attr, W1, b1, W2, b2, W3, b3):
    from concourse import bass_utils

    key = "prog"
    per_core, meta = _build_host_data(edge_index, edge_type, edge_attr)
    if key not in _CACHE:
        _CACHE[key] = _build_program(meta)
    nc = _CACHE[key]

    x = np.asarray(x, dtype=np.float32)
    deg = meta["deg"]
    Ws = np.stack([np.asarray(W1), np.asarray(W2), np.asarray(W3)]).astype(np.float32)
    Bs = np.stack([np.tile(np.asarray(b)[None, :], (128, 1))
                   for b in (b1, b2, b3)]).astype(np.float32)

    in_maps = []
    for k in range(NCORES):
        xblk = np.zeros((RPC, D), dtype=np.float32)
        xblk[:RPC_REAL] = x[k * RPC_REAL:(k + 1) * RPC_REAL]
        degw = np.zeros((128, NCHUNK), dtype=np.float32)
        dblk = np.zeros(RPC, dtype=np.float32)
        dblk[:RPC_REAL] = deg[k * RPC_REAL:(k + 1) * RPC_REAL]
        degw[:, :] = dblk.reshape(NCHUNK, 128).T
        idx16 = per_core[k]["idx16"]          # [cols, 16]
        idx_sb = np.tile(idx16.T, (8, 1))     # [128, cols]
        S = per_core[k]["S"]                  # [ntiles*WIN, 128] fp32
        import ml_dtypes
        S_bf = S.T.astype(ml_dtypes.bfloat16)  # [128, ntiles*WIN]
        in_maps.append({
            "x_in": xblk,
            "deg_in": degw,
            "idx_in": idx_sb,
            "s_in": S_bf,
            "w_in": Ws,
            "b_in": Bs,
        })

    trace = bool(int(os.environ.get("KTRACE", "0")))
    res = bass_utils.run_bass_kernel_spmd(nc, in_maps, core_ids=list(range(NCORES)),
                                          trace=trace)
    global _LAST_EXEC_NS, _LAST_RES
    _LAST_EXEC_NS = res.exec_time_ns
    _LAST_RES = res
    out = np.zeros((N, D), dtype=np.float32)
    for k in range(NCORES):
        out[k * RPC_REAL:(k + 1) * RPC_REAL] = res.results[k]["out_d"][:RPC_REAL]
    return out



# revision 3
# speedup vs baseline: 1.9977x; 1.9977x over previous
"""Trainium2 Bass kernel for 3-layer relational GCN message passing.

Math (per layer, from the reference):
  deg[c]  = #edges with col==c          (fixed across layers)
  dis     = where(deg>0, deg^-0.5, 0)
  out[r]  = leakyrelu( dis[r] * sum_t (sum_{e: row=r,type=t} w_e * (dis*x)[col_e]) @ W_t + b )
final = (x + z1 + z2 + z3) / 4

Strategy (8 NeuronCores, node-ownership sharding):
  - Core k owns rows [6250k, 6250(k+1)); every edge lives on the core owning
    its destination row.  Per layer each core gathers source vectors for its
    edges from a replicated bf16 table in DRAM (dma_gather, alternating two
    SWDGE queues so descriptor generation overlaps 2-way), scatter-reduces
    them into 512-dest PSUM windows via PE matmuls against host-built
    edge-weight one-hot "S" strips, then runs the 4-type GEMM per row chunk
    directly off the evicted window (feat-major), pipelined window-by-window.
  - Window = one row chunk of 128 rows x 4 edge types (dest = rc*512 +
    t*128 + r%128), so the GEMM for chunk rc starts as soon as window rc is
    scattered.
  - The node-feature table is split in two halves (A = each core's rows
    [0,3072), B = rows [3072,6272)) exchanged with two AllGathers per layer
    so the next layer's stream-A gathers start before stream B finishes.
  - Edge slots are padded only to the per-(window,stream) max across cores
    (~5% padding vs 28% for per-64-window tiles).
"""

import os
import sys
import numpy as np

sys.path.insert(0, "/opt/trn_rl_repo")

N = 50000
D = 128
NT = 4
NCORES = 8
RPC_REAL = 6250          # real rows per core
RPC = 6272               # padded rows per core (49*128)
NCHUNK = 49              # row chunks of 128 per core
NW = NCHUNK              # one 512-dest window per row chunk
WIN = 512
SPLITA = 3072            # per-core rows in table half A
SPLITB = RPC - SPLITA    # 3200
HA = NCORES * SPLITA     # 24576 rows in half A
HB = NCORES * SPLITB     # 25600 rows in half B
CT = 40                  # gather chunk size in 128-slot tiles
NEG_SLOPE = 0.01

_CACHE = {}


def _build_host_data(edge_index, edge_type, edge_attr):
    """Edge bookkeeping: per-core slot lists, int16 gather indices, S strips,
    and the static (uniform across cores) schedule."""
    row = edge_index[0].astype(np.int64)
    col = edge_index[1].astype(np.int64)
    et = edge_type.astype(np.int64)
    w = edge_attr.astype(np.float32)

    deg = np.bincount(col, minlength=N).astype(np.float32)

    owner = row // RPC_REAL
    r_loc = row - owner * RPC_REAL
    rc = r_loc // 128
    dest = rc * WIN + et * 128 + (r_loc % 128)   # within-core dest
    co = col // RPC_REAL
    c_loc = col - co * RPC_REAL
    s = (c_loc >= SPLITA).astype(np.int64)       # stream 0=A, 1=B
    tcol = np.where(s == 1, co * SPLITB + (c_loc - SPLITA), co * SPLITA + c_loc)

    # sort by (owner, stream, window, dest)
    order = np.lexsort((dest, rc, s, owner))
    dest_s, rc_s, s_s, own_s = dest[order], rc[order], s[order], owner[order]
    tcol_s, w_s = tcol[order], w[order]

    key = (own_s * 2 + s_s) * NW + rc_s
    cnt = np.bincount(key, minlength=NCORES * 2 * NW).reshape(NCORES, 2, NW)
    starts_flat = np.zeros(NCORES * 2 * NW + 1, dtype=np.int64)
    np.cumsum(cnt.reshape(-1), out=starts_flat[1:])

    u = cnt.max(axis=0)                          # [2, NW] padded bucket sizes
    ustart = np.zeros((2, NW), dtype=np.int64)   # slot offset of bucket
    U = np.zeros(2, dtype=np.int64)              # padded slots per stream
    ntile = np.zeros(2, dtype=np.int64)
    for st in range(2):
        ustart[st] = np.cumsum(u[st]) - u[st]
        U[st] = u[st].sum()
        ntile[st] = -(-U[st] // 128)

    # chunks: consecutive tiles
    chunks = []                                  # (stream, tile_lo, tile_hi)
    for st in range(2):
        t0 = 0
        while t0 < ntile[st]:
            t1 = min(t0 + CT, int(ntile[st]))
            chunks.append((st, t0, t1))
            t0 = t1
    # order chunks A0,B0,A1,B1,... for queue pairing
    ca = [c for c in chunks if c[0] == 0]
    cb = [c for c in chunks if c[0] == 1]
    chunks = []
    for i in range(max(len(ca), len(cb))):
        if i < len(ca):
            chunks.append(ca[i])
        if i < len(cb):
            chunks.append(cb[i])

    # per-(stream,tile) window strips: list of (rc, lo, width)
    # plus first/last bookkeeping per window for PSUM start/stop flags.
    # Slot -> core-local dest arrays for strip extents:
    dloc_s = dest_s - rc_s * WIN                 # 0..511 within window

    strips = [[] for _ in range(2)]              # per stream: per tile list
    for st in range(2):
        for t in range(int(ntile[st])):
            lo_sl, hi_sl = t * 128, t * 128 + 128
            # windows overlapping [lo_sl, hi_sl)
            tl = []
            wlo = int(np.searchsorted(ustart[st], lo_sl, side="right")) - 1
            whi = int(np.searchsorted(ustart[st], hi_sl, side="left"))
            for rcw in range(max(wlo, 0), min(whi, NW)):
                a = max(lo_sl, int(ustart[st][rcw]))
                b = min(hi_sl, int(ustart[st][rcw] + u[st][rcw]))
                if a >= b:
                    continue
                # strip extent = union over cores of dest range of real edges
                lo_d, hi_d = WIN, 0
                for k in range(NCORES):
                    base = starts_flat[(k * 2 + st) * NW + rcw]
                    nn = int(cnt[k, st, rcw])
                    e0 = a - int(ustart[st][rcw])
                    e1 = min(b - int(ustart[st][rcw]), nn)
                    if e0 >= e1:
                        continue
                    dd = dloc_s[base + e0: base + e1]
                    lo_d = min(lo_d, int(dd[0]))
                    hi_d = max(hi_d, int(dd[-1]) + 1)
                if lo_d >= hi_d:
                    lo_d, hi_d = 0, 1
                tl.append([rcw, lo_d, hi_d])
            strips[st].append(tl)

    # global emission order of scatter matmuls: by window rc, stream A strips
    # then B strips, tiles ascending.  first strip of each window is widened
    # to [0, WIN) (PSUM zero-init), flags start/stop assigned.
    win_strips = [[] for _ in range(NW)]         # (st, tile, lo, hi)
    for st in range(2):
        for t, tl in enumerate(strips[st]):
            for rcw, lo_d, hi_d in tl:
                win_strips[rcw].append([st, t, lo_d, hi_d])
    for rcw in range(NW):
        wl = sorted(win_strips[rcw], key=lambda x: (x[0], x[1]))
        wl[0][2], wl[0][3] = 0, WIN              # first strip full width
        win_strips[rcw] = wl

    # S column layout: strips stored per (stream, chunk) contiguously in
    # (tile, window) order; col offsets are uniform across cores.
    scol = {}                                    # (st, t, rcw) -> col offset
    chunk_scols = []                             # per chunk: (col_lo, col_hi)
    ccol = 0
    for (st, t0, t1) in chunks:
        c_lo = ccol
        for t in range(t0, t1):
            for rcw, lo_d, hi_d in _strips_of(strips, win_strips, st, t):
                scol[(st, t, rcw)] = ccol
                ccol += hi_d - lo_d
        chunk_scols.append((c_lo, ccol))
    stot = ccol

    # per-core data: idx arrays and S matrix
    per_core = []
    for k in range(NCORES):
        idxs = []
        for st in range(2):
            arr = np.zeros(int(ntile[st]) * 128, dtype=np.int16)
            for rcw in range(NW):
                base = starts_flat[(k * 2 + st) * NW + rcw]
                nn = int(cnt[k, st, rcw])
                off = int(ustart[st][rcw])
                arr[off:off + nn] = tcol_s[base:base + nn].astype(np.int16)
            idxs.append(arr)
        S = np.zeros((stot, 128), dtype=np.float32)
        for st in range(2):
            for t in range(int(ntile[st])):
                for rcw, lo_d, hi_d in _strips_of(strips, win_strips, st, t):
                    c0 = scol[(st, t, rcw)]
                    base = starts_flat[(k * 2 + st) * NW + rcw]
                    nn = int(cnt[k, st, rcw])
                    a = max(t * 128, int(ustart[st][rcw]))
                    b = min(t * 128 + 128, int(ustart[st][rcw]) + nn)
                    for sl in range(a, b):
                        e = base + (sl - int(ustart[st][rcw]))
                        d = int(dloc_s[e])
                        S[c0 + d - lo_d, sl - t * 128] = w_s[e]
        per_core.append({"idx": idxs, "S": S})

    meta = {
        "deg": deg, "u": u, "ustart": ustart, "U": U, "ntile": ntile,
        "chunks": chunks, "strips": strips, "win_strips": win_strips,
        "scol": scol, "chunk_scols": chunk_scols, "stot": stot,
    }
    return per_core, meta


def _strips_of(strips, win_strips, st, t):
    """Strips of tile t in stream st with final (lo, hi) after widening."""
    out = []
    for rcw, _lo, _hi in strips[st][t]:
        for st2, t2, lo2, hi2 in win_strips[rcw]:
            if st2 == st and t2 == t:
                out.append((rcw, lo2, hi2))
                break
    return out


def _build_program(meta):
    from concourse import bacc, mybir, tile

    dt = mybir.dt
    Afn = mybir.ActivationFunctionType
    Alu = mybir.AluOpType

    u = meta["u"]
    ustart = meta["ustart"]
    ntile = meta["ntile"]
    chunks = meta["chunks"]
    strips = meta["strips"]
    win_strips = meta["win_strips"]
    scol = meta["scol"]
    chunk_scols = meta["chunk_scols"]
    stot = meta["stot"]
    max_ccols = max(c1 - c0 for c0, c1 in chunk_scols)
    idx_cols = [int(ntile[0]) * 8, int(ntile[1]) * 8]   # 128 slots = 8 cols

    nc = bacc.Bacc("TRN2", target_bir_lowering=False, debug=False,
                   num_devices=NCORES, num_swdge_queues=2)

    x_in = nc.dram_tensor("x_in", [RPC, D], dt.float32, kind="ExternalInput").ap()
    deg_in = nc.dram_tensor("deg_in", [128, NCHUNK], dt.float32,
                            kind="ExternalInput").ap()
    idxa_in = nc.dram_tensor("idxa_in", [128, idx_cols[0]], dt.int16,
                             kind="ExternalInput").ap()
    idxb_in = nc.dram_tensor("idxb_in", [128, idx_cols[1]], dt.int16,
                             kind="ExternalInput").ap()
    s_in = nc.dram_tensor("s_in", [128, stot], dt.bfloat16,
                          kind="ExternalInput").ap()
    w_in = nc.dram_tensor("w_in", [3, NT, D, D], dt.float32,
                          kind="ExternalInput").ap()
    b_in = nc.dram_tensor("b_in", [3, 128, D], dt.float32,
                          kind="ExternalInput").ap()
    out_d = nc.dram_tensor("out_d", [RPC, D], dt.float32,
                           kind="ExternalOutput").ap()

    rg = [list(range(NCORES))]

    with tile.TileContext(nc) as tc:
        with tc.tile_pool(name="sb", bufs=1) as sb, \
             tc.tile_pool(name="gpool", bufs=4) as gpool, \
             tc.tile_pool(name="spool", bufs=3) as spool, \
             tc.tile_pool(name="mpool", bufs=3) as mpool, \
             tc.tile_pool(name="stage", bufs=4) as stage, \
             tc.tile_pool(name="psw", bufs=3, space="PSUM") as psw, \
             tc.tile_pool(name="psg", bufs=2, space="PSUM") as psg, \
             tc.tile_pool(name="dram", bufs=1, space="DRAM") as dram:

            # ---- persistent SBUF ----
            idx_sb = [sb.tile([128, idx_cols[0]], dt.int16, name="idxA"),
                      sb.tile([128, idx_cols[1]], dt.int16, name="idxB")]
            nc.sync.dma_start(idx_sb[0][:], idxa_in[:])
            nc.sync.dma_start(idx_sb[1][:], idxb_in[:])
            dis = sb.tile([128, NCHUNK], dt.float32)
            degt = sb.tile([128, NCHUNK], dt.float32)
            nc.sync.dma_start(degt[:], deg_in[:])
            W_sb = sb.tile([128, 3, NT, D], dt.bfloat16)
            for l in range(3):
                for t in range(NT):
                    wtmp = stage.tile([128, D], dt.float32, tag="wtmp")
                    nc.sync.dma_start(wtmp[:], w_in[l, t, :, :])
                    nc.vector.tensor_copy(W_sb[:, l, t, :], wtmp[:])
            B_sb = sb.tile([128, 3, D], dt.float32)
            for l in range(3):
                btmp = stage.tile([128, D], dt.float32, tag="wtmp")
                nc.sync.dma_start(btmp[:], b_in[l, :, :])
                nc.vector.tensor_copy(B_sb[:, l, :], btmp[:])
            acc = sb.tile([128, NCHUNK, D], dt.float32)

            # dis = where(deg>0, 1/sqrt(deg), 0)
            deg1 = stage.tile([128, NCHUNK], dt.float32, tag="d1")
            nc.vector.tensor_scalar_max(deg1[:], degt[:], 1.0)
            inv = stage.tile([128, NCHUNK], dt.float32, tag="d2")
            nc.vector.reciprocal(inv[:], deg1[:])
            sq = stage.tile([128, NCHUNK], dt.float32, tag="d3")
            nc.scalar.activation(sq[:], inv[:], Afn.Sqrt)
            mask = stage.tile([128, NCHUNK], dt.float32, tag="d4")
            nc.vector.tensor_scalar(mask[:], degt[:], 0.0, None, op0=Alu.is_gt)
            nc.vector.tensor_tensor(dis[:], sq[:], mask[:], op=Alu.mult)

            # ---- DRAM tables and per-layer input blocks ----
            tabA = [dram.tile([HA, D], dt.bfloat16, name=f"tabA{i}",
                              addr_space="Shared") for i in range(3)]
            tabB = [dram.tile([HB, D], dt.bfloat16, name=f"tabB{i}",
                              addr_space="Shared") for i in range(3)]
            blkA = [dram.tile([SPLITA, D], dt.bfloat16, name=f"blkA{i}")
                    for i in range(3)]
            blkB = [dram.tile([SPLITB, D], dt.bfloat16, name=f"blkB{i}")
                    for i in range(3)]

            # ---- prologue: acc = x ; blk0 = dis * x ; AllGather halves ----
            for rcc in range(NCHUNK):
                xc = stage.tile([128, D], dt.float32, tag="xc")
                nc.sync.dma_start(xc[:], x_in[rcc * 128:(rcc + 1) * 128, :])
                nc.vector.tensor_copy(acc[:, rcc, :], xc[:])
                zt = stage.tile([128, D], dt.bfloat16, tag="zt")
                nc.scalar.activation(zt[:], xc[:], Afn.Copy,
                                     scale=dis[:, rcc:rcc + 1])
                if rcc < SPLITA // 128:
                    nc.sync.dma_start(blkA[0][rcc * 128:(rcc + 1) * 128, :], zt[:])
                else:
                    r2 = rcc - SPLITA // 128
                    nc.sync.dma_start(blkB[0][r2 * 128:(r2 + 1) * 128, :], zt[:])
                if rcc == SPLITA // 128 - 1:
                    nc.gpsimd.collective_compute(
                        "AllGather", Alu.bypass, replica_groups=rg,
                        ins=[blkA[0].opt()], outs=[tabA[0].opt()])
            nc.gpsimd.collective_compute(
                "AllGather", Alu.bypass, replica_groups=rg,
                ins=[blkB[0].opt()], outs=[tabB[0].opt()])

            # ---- layers ----
            gq = 0  # gather queue round-robin
            for l in range(3):
                table = [tabA[l], tabB[l]]
                # per-stream chunk tiles currently resident: dict chunk->tile
                gts = {}
                sts_ = {}
                emitted = [0, 0]            # next chunk index per stream
                chunk_of = [{}, {}]         # (stream, tile) -> chunk id
                for ci, (st, t0, t1) in enumerate(chunks):
                    for t in range(t0, t1):
                        chunk_of[st][t] = ci

                def emit_chunk(ci):
                    nonlocal gq
                    st, t0, t1 = chunks[ci]
                    nb = t1 - t0
                    gt = gpool.tile([128, CT, D], dt.bfloat16, tag="gt")
                    nc.gpsimd.dma_gather(
                        out_ap=gt[:, :nb, :],
                        in_ap=table[st][:],
                        idxs_ap=idx_sb[st][:, t0 * 8: t1 * 8],
                        num_idxs=nb * 128,
                        num_idxs_reg=nb * 128,
                        elem_size=D,
                        single_packet=False,
                        queue_num=gq,
                    )
                    gq ^= 1
                    c0, c1 = chunk_scols[ci]
                    ssb = spool.tile([128, max_ccols], dt.bfloat16, tag="ssb")
                    nc.sync.dma_start(ssb[:, :c1 - c0], s_in[:, c0:c1])
                    gts[ci] = gt
                    sts_[ci] = (ssb, c0)

                # which chunks must be emitted before window rcw's matmuls
                need_chunk = [[0, 0] for _ in range(NW)]
                for rcw in range(NW):
                    for st in range(2):
                        last_slot = int(ustart[st][rcw] + u[st][rcw]) - 1
                        need_chunk[rcw][st] = chunk_of[st][last_slot // 128]

                for rcw in range(NW):
                    for st in range(2):
                        # emit gather chunks up to the one containing rcw's end
                        nd = need_chunk[rcw][st]
                        for ci in range(len(chunks)):
                            if chunks[ci][0] == st and ci not in gts \
                                    and _chunk_ord(chunks, st, ci) <= \
                                    _chunk_ord(chunks, st, nd):
                                emit_chunk(ci)
                    # scatter matmuls for window rcw
                    pw = psw.tile([128, WIN], dt.float32, tag="pw")
                    wl = win_strips[rcw]
                    for i, (st, t, lo_d, hi_d) in enumerate(wl):
                        ci = chunk_of[st][t]
                        gt = gts[ci]
                        ssb, c0 = sts_[ci]
                        so = scol[(st, t, rcw)] - c0
                        tloc = t - chunks[ci][1]
                        nc.tensor.matmul(
                            out=pw[:, lo_d:hi_d],
                            lhsT=gt[:, t - chunks[ci][1], :],
                            rhs=ssb[:, so:so + hi_d - lo_d],
                            start=(i == 0),
                            stop=(i == len(wl) - 1),
                        )
                    # evict window -> msg bf16
                    msg = mpool.tile([128, WIN], dt.bfloat16, tag="msg")
                    nc.scalar.activation(msg[:], pw[:], Afn.Copy)
                    # GEMM for row chunk rcw
                    pg = psg.tile([128, D], dt.float32, tag="pg")
                    for t in range(NT):
                        nc.tensor.matmul(
                            out=pg[:],
                            lhsT=msg[:, t * 128:(t + 1) * 128],
                            rhs=W_sb[:, l, t, :],
                            start=(t == 0),
                            stop=(t == NT - 1),
                        )
                    tz = stage.tile([128, D], dt.float32, tag="tz")
                    nc.scalar.activation(tz[:], pg[:], Afn.Copy,
                                         scale=dis[:, rcw:rcw + 1])
                    nc.vector.tensor_tensor(tz[:], tz[:], B_sb[:, l, :],
                                            op=Alu.add)
                    z = stage.tile([128, D], dt.float32, tag="z")
                    nc.scalar.activation(z[:], tz[:], Afn.Lrelu,
                                         alpha=NEG_SLOPE)
                    if l < 2:
                        nc.vector.tensor_tensor(acc[:, rcw, :], acc[:, rcw, :],
                                                z[:], op=Alu.add)
                        zt = stage.tile([128, D], dt.bfloat16, tag="zt2")
                        nc.scalar.activation(zt[:], z[:], Afn.Copy,
                                             scale=dis[:, rcw:rcw + 1])
                        if rcw < SPLITA // 128:
                            nc.sync.dma_start(
                                blkA[l + 1][rcw * 128:(rcw + 1) * 128, :], zt[:])
                        else:
                            r2 = rcw - SPLITA // 128
                            nc.sync.dma_start(
                                blkB[l + 1][r2 * 128:(r2 + 1) * 128, :], zt[:])
                        if rcw == 40:
                            nc.gpsimd.collective_compute(
                                "AllGather", Alu.bypass, replica_groups=rg,
                                ins=[blkA[l + 1].opt()],
                                outs=[tabA[l + 1].opt()])
                    else:
                        # final layer: out = (acc + z) / 4
                        oc = stage.tile([128, D], dt.float32, tag="oc")
                        nc.vector.tensor_tensor(oc[:], acc[:, rcw, :], z[:],
                                                op=Alu.add)
                        o2 = stage.tile([128, D], dt.float32, tag="o2")
                        nc.vector.tensor_scalar_mul(o2[:], oc[:], 0.25)
                        nc.sync.dma_start(out_d[rcw * 128:(rcw + 1) * 128, :],
                                          o2[:])
                if l < 2:
                    nc.gpsimd.collective_compute(
                        "AllGather", Alu.bypass, replica_groups=rg,
                        ins=[blkB[l + 1].opt()], outs=[tabB[l + 1].opt()])

    nc.compile()
    return nc


def _chunk_ord(chunks, st, ci):
    """Ordinal of chunk ci within its stream."""
    n = 0
    for j in range(ci):
        if chunks[j][0] == st:
            n += 1
    return n


def kernel(x, edge_index, edge_type, edge_attr, W1, b1, W2, b2, W3, b3):
    from concourse import bass_utils
    import ml_dtypes

    key = "prog"
    per_core, meta = _build_host_data(edge_index, edge_type, edge_attr)
    if key not in _CACHE:
        _CACHE[key] = _build_program(meta)
    nc = _CACHE[key]

    x = np.asarray(x, dtype=np.float32)
    deg = meta["deg"]
    Ws = np.stack([np.asarray(W1), np.asarray(W2), np.asarray(W3)]).astype(np.float32)
    Bs = np.stack([np.tile(np.asarray(b)[None, :], (128, 1))
                   for b in (b1, b2, b3)]).astype(np.float32)

    in_maps = []
    for k in range(NCORES):
        xblk = np.zeros((RPC, D), dtype=np.float32)
        xblk[:RPC_REAL] = x[k * RPC_REAL:(k + 1) * RPC_REAL]
        degw = np.zeros((128, NCHUNK), dtype=np.float32)
        dblk = np.zeros(RPC, dtype=np.float32)
        dblk[:RPC_REAL] = deg[k * RPC_REAL:(k + 1) * RPC_REAL]
        degw[:, :] = dblk.reshape(NCHUNK, 128).T
        idxs = per_core[k]["idx"]
        idx_maps = []
        for st in range(2):
            arr = idxs[st].reshape(-1, 16)        # [cols, 16]
            idx_maps.append(np.tile(arr.T, (8, 1)))  # [128, cols]
        S_bf = per_core[k]["S"].T.astype(ml_dtypes.bfloat16)  # [128, stot]
        in_maps.append({
            "x_in": xblk,
            "deg_in": degw,
            "idxa_in": idx_maps[0],
            "idxb_in": idx_maps[1],
            "s_in": S_bf,
            "w_in": Ws,
            "b_in": Bs,
        })

    trace = bool(int(os.environ.get("KTRACE", "0")))
    res = bass_utils.run_bass_kernel_spmd(nc, in_maps, core_ids=list(range(NCORES)),
                                          trace=trace)
    global _LAST_EXEC_NS, _LAST_RES
    _LAST_EXEC_NS = res.exec_time_ns
    _LAST_RES = res
    out = np.zeros((N, D), dtype=np.float32)
    for k in range(NCORES):
        out[k * RPC_REAL:(k + 1) * RPC_REAL] = res.results[k]["out_d"][:RPC_REAL]
    return out


# revision 9
# speedup vs baseline: 2.2027x; 1.1026x over previous
"""Trainium2 Bass kernel for 3-layer relational GCN message passing.

Math (per layer, from the reference):
  deg[c]  = #edges with col==c          (fixed across layers)
  dis     = where(deg>0, deg^-0.5, 0)
  out[r]  = leakyrelu( dis[r] * sum_t (sum_{e: row=r,type=t} w_e * (dis*x)[col_e]) @ W_t + b )
final = (x + z1 + z2 + z3) / 4

Strategy (8 NeuronCores, node-ownership sharding):
  - Core k owns rows [6250k, 6250(k+1)); every edge lives on the core owning
    its destination row.  Per layer each core gathers source vectors for its
    edges from a replicated bf16 table in DRAM (dma_gather, alternating two
    SWDGE queues so descriptor generation overlaps 2-way), scatter-reduces
    them into 512-dest PSUM windows via PE matmuls against host-built
    edge-weight one-hot "S" strips, then runs the 4-type GEMM per row chunk
    directly off the evicted window (feat-major), pipelined window-by-window.
  - Window = one row chunk of 128 rows x 4 edge types (dest = rc*512 +
    t*128 + r%128), so the GEMM for chunk rc starts as soon as window rc is
    scattered.
  - The node-feature table is split in two halves (A = each core's rows
    [0,3072), B = rows [3072,6272)) exchanged with two AllGathers per layer
    so the next layer's stream-A gathers start before stream B finishes.
  - Edge slots are padded only to the per-(window,stream) max across cores
    (~5% padding vs 28% for per-64-window tiles).
"""

import os
import sys
import numpy as np

sys.path.insert(0, "/opt/trn_rl_repo")

N = 50000
D = 128
NT = 4
NCORES = 8
RPC_REAL = 6250          # real rows per core
RPC = 6272               # padded rows per core (49*128)
NCHUNK = 49              # row chunks of 128 per core
NW = NCHUNK              # one 512-dest window per row chunk
WIN = 512
SPLITA = 3072            # per-core rows in table half A
SPLITB = RPC - SPLITA    # 3200
HA = NCORES * SPLITA     # 24576 rows in half A
HB = NCORES * SPLITB     # 25600 rows in half B
CT = 40                  # gather chunk size in 128-slot tiles
NEG_SLOPE = 0.01

_CACHE = {}


def _build_host_data(edge_index, edge_type, edge_attr):
    """Edge bookkeeping: per-core slot lists, int16 gather indices, S strips,
    and the static (uniform across cores) schedule."""
    row = edge_index[0].astype(np.int64)
    col = edge_index[1].astype(np.int64)
    et = edge_type.astype(np.int64)
    w = edge_attr.astype(np.float32)

    deg = np.bincount(col, minlength=N).astype(np.float32)

    owner = row // RPC_REAL
    r_loc = row - owner * RPC_REAL
    rc = r_loc // 128
    dest = rc * WIN + et * 128 + (r_loc % 128)   # within-core dest
    co = col // RPC_REAL
    c_loc = col - co * RPC_REAL
    s = (c_loc >= SPLITA).astype(np.int64)       # stream 0=A, 1=B
    tcol = np.where(s == 1, co * SPLITB + (c_loc - SPLITA), co * SPLITA + c_loc)

    # sort by (owner, stream, window, dest)
    order = np.lexsort((dest, rc, s, owner))
    dest_s, rc_s, s_s, own_s = dest[order], rc[order], s[order], owner[order]
    tcol_s, w_s = tcol[order], w[order]

    key = (own_s * 2 + s_s) * NW + rc_s
    cnt = np.bincount(key, minlength=NCORES * 2 * NW).reshape(NCORES, 2, NW)
    starts_flat = np.zeros(NCORES * 2 * NW + 1, dtype=np.int64)
    np.cumsum(cnt.reshape(-1), out=starts_flat[1:])

    u = cnt.max(axis=0)                          # [2, NW] padded bucket sizes
    ustart = np.zeros((2, NW), dtype=np.int64)   # slot offset of bucket
    U = np.zeros(2, dtype=np.int64)              # padded slots per stream
    ntile = np.zeros(2, dtype=np.int64)
    for st in range(2):
        ustart[st] = np.cumsum(u[st]) - u[st]
        U[st] = u[st].sum()
        ntile[st] = -(-U[st] // 128)

    # chunks: consecutive tiles
    chunks = []                                  # (stream, tile_lo, tile_hi)
    for st in range(2):
        t0 = 0
        while t0 < ntile[st]:
            t1 = min(t0 + CT, int(ntile[st]))
            chunks.append((st, t0, t1))
            t0 = t1
    # order chunks A0,A1,B0,A2,B1,... (B delayed one slot: tab-B's AllGather
    # finishes after tab-A's, so the first B gather would otherwise stall)
    ca = [c for c in chunks if c[0] == 0]
    cb = [c for c in chunks if c[0] == 1]
    chunks = ca[:2]
    for i in range(max(len(ca) - 2, len(cb))):
        if i < len(cb):
            chunks.append(cb[i])
        if i + 2 < len(ca):
            chunks.append(ca[i + 2])

    # per-(stream,tile) window strips: list of (rc, lo, width)
    # plus first/last bookkeeping per window for PSUM start/stop flags.
    # Slot -> core-local dest arrays for strip extents:
    dloc_s = dest_s - rc_s * WIN                 # 0..511 within window

    strips = [[] for _ in range(2)]              # per stream: per tile list
    for st in range(2):
        for t in range(int(ntile[st])):
            lo_sl, hi_sl = t * 128, t * 128 + 128
            # windows overlapping [lo_sl, hi_sl)
            tl = []
            wlo = int(np.searchsorted(ustart[st], lo_sl, side="right")) - 1
            whi = int(np.searchsorted(ustart[st], hi_sl, side="left"))
            for rcw in range(max(wlo, 0), min(whi, NW)):
                a = max(lo_sl, int(ustart[st][rcw]))
                b = min(hi_sl, int(ustart[st][rcw] + u[st][rcw]))
                if a >= b:
                    continue
                # strip extent = union over cores of dest range of real edges
                lo_d, hi_d = WIN, 0
                for k in range(NCORES):
                    base = starts_flat[(k * 2 + st) * NW + rcw]
                    nn = int(cnt[k, st, rcw])
                    e0 = a - int(ustart[st][rcw])
                    e1 = min(b - int(ustart[st][rcw]), nn)
                    if e0 >= e1:
                        continue
                    dd = dloc_s[base + e0: base + e1]
                    lo_d = min(lo_d, int(dd[0]))
                    hi_d = max(hi_d, int(dd[-1]) + 1)
                if lo_d >= hi_d:
                    lo_d, hi_d = 0, 1
                tl.append([rcw, lo_d, hi_d])
            strips[st].append(tl)

    # global emission order of scatter matmuls: by window rc, stream A strips
    # then B strips, tiles ascending.  first strip of each window is widened
    # to [0, WIN) (PSUM zero-init), flags start/stop assigned.
    win_strips = [[] for _ in range(NW)]         # (st, tile, lo, hi)
    for st in range(2):
        for t, tl in enumerate(strips[st]):
            for rcw, lo_d, hi_d in tl:
                win_strips[rcw].append([st, t, lo_d, hi_d])
    for rcw in range(NW):
        wl = sorted(win_strips[rcw], key=lambda x: (x[0], x[1]))
        wl[0][2], wl[0][3] = 0, WIN              # first strip full width
        win_strips[rcw] = wl

    # S column layout: strips stored per (stream, chunk) contiguously in
    # (tile, window) order; col offsets are uniform across cores.
    scol = {}                                    # (st, t, rcw) -> col offset
    chunk_scols = []                             # per chunk: (col_lo, col_hi)
    ccol = 0
    for (st, t0, t1) in chunks:
        c_lo = ccol
        for t in range(t0, t1):
            for rcw, lo_d, hi_d in _strips_of(strips, win_strips, st, t):
                scol[(st, t, rcw)] = ccol
                ccol += hi_d - lo_d
        chunk_scols.append((c_lo, ccol))
    stot = ccol

    # per-core data: idx arrays and S matrix
    per_core = []
    for k in range(NCORES):
        idxs = []
        for st in range(2):
            arr = np.zeros(int(ntile[st]) * 128, dtype=np.int16)
            for rcw in range(NW):
                base = starts_flat[(k * 2 + st) * NW + rcw]
                nn = int(cnt[k, st, rcw])
                off = int(ustart[st][rcw])
                arr[off:off + nn] = tcol_s[base:base + nn].astype(np.int16)
            idxs.append(arr)
        S = np.zeros((stot, 128), dtype=np.float32)
        for st in range(2):
            for t in range(int(ntile[st])):
                for rcw, lo_d, hi_d in _strips_of(strips, win_strips, st, t):
                    c0 = scol[(st, t, rcw)]
                    base = starts_flat[(k * 2 + st) * NW + rcw]
                    nn = int(cnt[k, st, rcw])
                    a = max(t * 128, int(ustart[st][rcw]))
                    b = min(t * 128 + 128, int(ustart[st][rcw]) + nn)
                    for sl in range(a, b):
                        e = base + (sl - int(ustart[st][rcw]))
                        d = int(dloc_s[e])
                        S[c0 + d - lo_d, sl - t * 128] = w_s[e]
        per_core.append({"idx": idxs, "S": S})

    meta = {
        "deg": deg, "u": u, "ustart": ustart, "U": U, "ntile": ntile,
        "chunks": chunks, "strips": strips, "win_strips": win_strips,
        "scol": scol, "chunk_scols": chunk_scols, "stot": stot,
    }
    return per_core, meta


def _strips_of(strips, win_strips, st, t):
    """Strips of tile t in stream st with final (lo, hi) after widening."""
    out = []
    for rcw, _lo, _hi in strips[st][t]:
        for st2, t2, lo2, hi2 in win_strips[rcw]:
            if st2 == st and t2 == t:
                out.append((rcw, lo2, hi2))
                break
    return out


def _build_program(meta):
    from concourse import bacc, mybir, tile

    dt = mybir.dt
    Afn = mybir.ActivationFunctionType
    Alu = mybir.AluOpType

    u = meta["u"]
    ustart = meta["ustart"]
    ntile = meta["ntile"]
    chunks = meta["chunks"]
    strips = meta["strips"]
    win_strips = meta["win_strips"]
    scol = meta["scol"]
    chunk_scols = meta["chunk_scols"]
    stot = meta["stot"]
    max_ccols = max(c1 - c0 for c0, c1 in chunk_scols)
    idx_cols = [int(ntile[0]) * 8, int(ntile[1]) * 8]   # 128 slots = 8 cols

    nc = bacc.Bacc("TRN2", target_bir_lowering=False, debug=False,
                   num_devices=NCORES, num_swdge_queues=2)

    x_in = nc.dram_tensor("x_in", [RPC, D], dt.float32, kind="ExternalInput").ap()
    deg_in = nc.dram_tensor("deg_in", [128, NCHUNK], dt.float32,
                            kind="ExternalInput").ap()
    idxa_in = nc.dram_tensor("idxa_in", [128, idx_cols[0]], dt.int16,
                             kind="ExternalInput").ap()
    idxb_in = nc.dram_tensor("idxb_in", [128, idx_cols[1]], dt.int16,
                             kind="ExternalInput").ap()
    s_in = nc.dram_tensor("s_in", [128, stot], dt.bfloat16,
                          kind="ExternalInput").ap()
    w_in = nc.dram_tensor("w_in", [3, NT, D, D], dt.float32,
                          kind="ExternalInput").ap()
    b_in = nc.dram_tensor("b_in", [3, 128, D], dt.float32,
                          kind="ExternalInput").ap()
    out_d = nc.dram_tensor("out_d", [RPC, D], dt.float32,
                           kind="ExternalOutput").ap()

    rg = [list(range(NCORES))]

    with tile.TileContext(nc) as tc:
        with tc.tile_pool(name="sb", bufs=1) as sb, \
             tc.tile_pool(name="gpool", bufs=4) as gpool, \
             tc.tile_pool(name="spool", bufs=3) as spool, \
             tc.tile_pool(name="mpool", bufs=3) as mpool, \
             tc.tile_pool(name="stage", bufs=4) as stage, \
             tc.tile_pool(name="psw", bufs=3, space="PSUM") as psw, \
             tc.tile_pool(name="psg", bufs=2, space="PSUM") as psg, \
             tc.tile_pool(name="dram", bufs=1, space="DRAM") as dram:

            # ---- persistent SBUF ----
            idx_sb = [sb.tile([128, idx_cols[0]], dt.int16, name="idxA"),
                      sb.tile([128, idx_cols[1]], dt.int16, name="idxB")]
            nc.sync.dma_start(idx_sb[0][:], idxa_in[:])
            nc.sync.dma_start(idx_sb[1][:], idxb_in[:])
            dis = sb.tile([128, NCHUNK], dt.float32)
            degt = sb.tile([128, NCHUNK], dt.float32)
            nc.sync.dma_start(degt[:], deg_in[:])
            W_sb = sb.tile([128, 3, NT, D], dt.bfloat16)
            for l in range(3):
                for t in range(NT):
                    wtmp = stage.tile([128, D], dt.float32, tag="wtmp")
                    nc.sync.dma_start(wtmp[:], w_in[l, t, :, :])
                    nc.vector.tensor_copy(W_sb[:, l, t, :], wtmp[:])
            B_sb = sb.tile([128, 3, D], dt.float32)
            for l in range(3):
                btmp = stage.tile([128, D], dt.float32, tag="wtmp")
                nc.sync.dma_start(btmp[:], b_in[l, :, :])
                nc.vector.tensor_copy(B_sb[:, l, :], btmp[:])
            acc = sb.tile([128, NCHUNK, D], dt.float32)

            # dis = where(deg>0, 1/sqrt(deg), 0)
            deg1 = stage.tile([128, NCHUNK], dt.float32, tag="d1")
            nc.vector.tensor_scalar_max(deg1[:], degt[:], 1.0)
            inv = stage.tile([128, NCHUNK], dt.float32, tag="d2")
            nc.vector.reciprocal(inv[:], deg1[:])
            sq = stage.tile([128, NCHUNK], dt.float32, tag="d3")
            nc.scalar.activation(sq[:], inv[:], Afn.Sqrt)
            mask = stage.tile([128, NCHUNK], dt.float32, tag="d4")
            nc.vector.tensor_scalar(mask[:], degt[:], 0.0, None, op0=Alu.is_gt)
            nc.vector.tensor_tensor(dis[:], sq[:], mask[:], op=Alu.mult)
            # W/b are pre-scaled by 1/4 on the host (folds the final /4 into
            # each layer via lrelu homogeneity); table entries need dis*z =
            # 4*dis*z', so layer outputs are rescaled with dis4.
            dis4 = sb.tile([128, NCHUNK], dt.float32)
            nc.vector.tensor_scalar_mul(dis4[:], dis[:], 4.0)

            # ---- DRAM tables and per-layer input blocks ----
            tabA = [dram.tile([HA, D], dt.bfloat16, name=f"tabA{i}",
                              addr_space="Shared") for i in range(3)]
            tabB = [dram.tile([HB, D], dt.bfloat16, name=f"tabB{i}",
                              addr_space="Shared") for i in range(3)]
            blkA = [dram.tile([SPLITA, D], dt.bfloat16, name=f"blkA{i}")
                    for i in range(3)]
            blkB = [dram.tile([SPLITB, D], dt.bfloat16, name=f"blkB{i}")
                    for i in range(3)]

            # ---- prologue: acc = x ; blk0 = dis * x ; AllGather halves ----
            for rcc in range(NCHUNK):
                xc = stage.tile([128, D], dt.float32, tag="xc")
                nc.sync.dma_start(xc[:], x_in[rcc * 128:(rcc + 1) * 128, :])
                nc.vector.tensor_scalar_mul(acc[:, rcc, :], xc[:], 0.25)
                zt = stage.tile([128, D], dt.bfloat16, tag="zt")
                nc.scalar.activation(zt[:], xc[:], Afn.Copy,
                                     scale=dis[:, rcc:rcc + 1])
                if rcc < SPLITA // 128:
                    nc.sync.dma_start(blkA[0][rcc * 128:(rcc + 1) * 128, :], zt[:])
                else:
                    r2 = rcc - SPLITA // 128
                    nc.sync.dma_start(blkB[0][r2 * 128:(r2 + 1) * 128, :], zt[:])
                if rcc == SPLITA // 128 - 1:
                    nc.gpsimd.collective_compute(
                        "AllGather", Alu.bypass, replica_groups=rg,
                        ins=[blkA[0].opt()], outs=[tabA[0].opt()])
            nc.gpsimd.collective_compute(
                "AllGather", Alu.bypass, replica_groups=rg,
                ins=[blkB[0].opt()], outs=[tabB[0].opt()])

            # ---- layers ----
            gq = 0  # gather queue round-robin
            for l in range(3):
                table = [tabA[l], tabB[l]]
                # per-stream chunk tiles currently resident: dict chunk->tile
                gts = {}
                sts_ = {}
                chunk_of = [{}, {}]         # (stream, tile) -> chunk id
                for ci, (st, t0, t1) in enumerate(chunks):
                    for t in range(t0, t1):
                        chunk_of[st][t] = ci

                def emit_chunk(ci):
                    nonlocal gq
                    st, t0, t1 = chunks[ci]
                    nb = t1 - t0
                    gt = gpool.tile([128, CT, D], dt.bfloat16, tag="gt")
                    nc.gpsimd.dma_gather(
                        out_ap=gt[:, :nb, :],
                        in_ap=table[st][:],
                        idxs_ap=idx_sb[st][:, t0 * 8: t1 * 8],
                        num_idxs=nb * 128,
                        num_idxs_reg=nb * 128,
                        elem_size=D,
                        single_packet=False,
                        queue_num=gq,
                    )
                    gq ^= 1
                    c0, c1 = chunk_scols[ci]
                    ssb = spool.tile([128, max_ccols], dt.bfloat16, tag="ssb")
                    nc.sync.dma_start(ssb[:, :c1 - c0], s_in[:, c0:c1])
                    gts[ci] = gt
                    sts_[ci] = (ssb, c0)

                # which chunks must be emitted before window rcw's matmuls
                need_chunk = [[0, 0] for _ in range(NW)]
                for rcw in range(NW):
                    for st in range(2):
                        last_slot = int(ustart[st][rcw] + u[st][rcw]) - 1
                        need_chunk[rcw][st] = chunk_of[st][last_slot // 128]

                for rcw in range(NW):
                    for st in range(2):
                        # emit gather chunks up to the one containing rcw's end
                        nd = need_chunk[rcw][st]
                        for ci in range(len(chunks)):
                            if chunks[ci][0] == st and ci not in gts \
                                    and _chunk_ord(chunks, st, ci) <= \
                                    _chunk_ord(chunks, st, nd):
                                emit_chunk(ci)
                    # scatter matmuls for window rcw
                    pw = psw.tile([128, WIN], dt.float32, tag="pw")
                    wl = win_strips[rcw]
                    for i, (st, t, lo_d, hi_d) in enumerate(wl):
                        ci = chunk_of[st][t]
                        gt = gts[ci]
                        ssb, c0 = sts_[ci]
                        so = scol[(st, t, rcw)] - c0
                        nc.tensor.matmul(
                            out=pw[:, lo_d:hi_d],
                            lhsT=gt[:, t - chunks[ci][1], :],
                            rhs=ssb[:, so:so + hi_d - lo_d],
                            start=(i == 0),
                            stop=(i == len(wl) - 1),
                        )
                    # evict window -> msg bf16
                    msg = mpool.tile([128, WIN], dt.bfloat16, tag="msg")
                    nc.scalar.activation(msg[:], pw[:], Afn.Copy)
                    # GEMM for row chunk rcw
                    pg = psg.tile([128, D], dt.float32, tag="pg")
                    for t in range(NT):
                        nc.tensor.matmul(
                            out=pg[:],
                            lhsT=msg[:, t * 128:(t + 1) * 128],
                            rhs=W_sb[:, l, t, :],
                            start=(t == 0),
                            stop=(t == NT - 1),
                        )
                    tz = stage.tile([128, D], dt.float32, tag="tz")
                    nc.scalar.activation(tz[:], pg[:], Afn.Copy,
                                         scale=dis[:, rcw:rcw + 1])
                    nc.vector.tensor_tensor(tz[:], tz[:], B_sb[:, l, :],
                                            op=Alu.add)
                    z = stage.tile([128, D], dt.float32, tag="z")
                    nc.scalar.activation(z[:], tz[:], Afn.Lrelu,
                                         alpha=NEG_SLOPE)
                    if l < 2:
                        nc.vector.tensor_tensor(acc[:, rcw, :], acc[:, rcw, :],
                                                z[:], op=Alu.add)
                        zt = stage.tile([128, D], dt.bfloat16, tag="zt2")
                        nc.scalar.activation(zt[:], z[:], Afn.Copy,
                                             scale=dis4[:, rcw:rcw + 1])
                        if rcw < SPLITA // 128:
                            nc.sync.dma_start(
                                blkA[l + 1][rcw * 128:(rcw + 1) * 128, :], zt[:])
                        else:
                            r2 = rcw - SPLITA // 128
                            nc.sync.dma_start(
                                blkB[l + 1][r2 * 128:(r2 + 1) * 128, :], zt[:])
                        if rcw == 26:
                            nc.gpsimd.collective_compute(
                                "AllGather", Alu.bypass, replica_groups=rg,
                                ins=[blkA[l + 1].opt()],
                                outs=[tabA[l + 1].opt()])
                    else:
                        # final layer: out = acc + z' (the /4 is pre-folded
                        # into W/b, and acc accumulates 0.25-scaled terms)
                        oc = stage.tile([128, D], dt.float32, tag="oc")
                        nc.vector.tensor_tensor(oc[:], acc[:, rcw, :], z[:],
                                                op=Alu.add)
                        nc.sync.dma_start(out_d[rcw * 128:(rcw + 1) * 128, :],
                                          oc[:])
                if l < 2:
                    nc.gpsimd.collective_compute(
                        "AllGather", Alu.bypass, replica_groups=rg,
                        ins=[blkB[l + 1].opt()], outs=[tabB[l + 1].opt()])

    nc.compile()
    return nc


def _chunk_ord(chunks, st, ci):
    """Ordinal of chunk ci within its stream."""
    n = 0
    for j in range(ci):
        if chunks[j][0] == st:
            n += 1
    return n


def kernel(x, edge_index, edge_type, edge_attr, W1, b1, W2, b2, W3, b3):
    from concourse import bass_utils
    import ml_dtypes

    key = "prog"
    per_core, meta = _build_host_data(edge_index, edge_type, edge_attr)
    if key not in _CACHE:
        _CACHE[key] = _build_program(meta)
    nc = _CACHE[key]

    x = np.asarray(x, dtype=np.float32)
    deg = meta["deg"]
    # final /4 folded into each layer: lrelu(0.25*(pre)) = 0.25*lrelu(pre)
    Ws = 0.25 * np.stack([np.asarray(W1), np.asarray(W2),
                          np.asarray(W3)]).astype(np.float32)
    Bs = 0.25 * np.stack([np.tile(np.asarray(b)[None, :], (128, 1))
                          for b in (b1, b2, b3)]).astype(np.float32)

    in_maps = []
    for k in range(NCORES):
        xblk = np.zeros((RPC, D), dtype=np.float32)
        xblk[:RPC_REAL] = x[k * RPC_REAL:(k + 1) * RPC_REAL]
        degw = np.zeros((128, NCHUNK), dtype=np.float32)
        dblk = np.zeros(RPC, dtype=np.float32)
        dblk[:RPC_REAL] = deg[k * RPC_REAL:(k + 1) * RPC_REAL]
        degw[:, :] = dblk.reshape(NCHUNK, 128).T
        idxs = per_core[k]["idx"]
        idx_maps = []
        for st in range(2):
            arr = idxs[st].reshape(-1, 16)        # [cols, 16]
            idx_maps.append(np.tile(arr.T, (8, 1)))  # [128, cols]
        S_bf = per_core[k]["S"].T.astype(ml_dtypes.bfloat16)  # [128, stot]
        in_maps.append({
            "x_in": xblk,
            "deg_in": degw,
            "idxa_in": idx_maps[0],
            "idxb_in": idx_maps[1],
            "s_in": S_bf,
            "w_in": Ws,
            "b_in": Bs,
        })

    trace = bool(int(os.environ.get("KTRACE", "0")))
    res = bass_utils.run_bass_kernel_spmd(nc, in_maps, core_ids=list(range(NCORES)),
                                          trace=trace)
    global _LAST_EXEC_NS, _LAST_RES
    _LAST_EXEC_NS = res.exec_time_ns
    _LAST_RES = res
    out = np.zeros((N, D), dtype=np.float32)
    for k in range(NCORES):
        out[k * RPC_REAL:(k + 1) * RPC_REAL] = res.results[k]["out_d"][:RPC_REAL]
    return out


# revision 15
# speedup vs baseline: 2.2707x; 1.0309x over previous
"""Trainium2 Bass kernel for 3-layer relational GCN message passing.

Math (per layer, from the reference):
  deg[c]  = #edges with col==c          (fixed across layers)
  dis     = where(deg>0, deg^-0.5, 0)
  out[r]  = leakyrelu( dis[r] * sum_t (sum_{e: row=r,type=t} w_e * (dis*x)[col_e]) @ W_t + b )
final = (x + z1 + z2 + z3) / 4

Strategy (8 NeuronCores, node-ownership sharding):
  - Core k owns rows [6250k, 6250(k+1)); every edge lives on the core owning
    its destination row.  Per layer each core gathers source vectors for its
    edges from a replicated bf16 table in DRAM (dma_gather, alternating two
    SWDGE queues so descriptor generation overlaps 2-way), scatter-reduces
    them into 512-dest PSUM windows via PE matmuls against host-built
    edge-weight one-hot "S" strips, then runs the 4-type GEMM per row chunk
    directly off the evicted window (feat-major), pipelined window-by-window.
  - Window = one row chunk of 128 rows x 4 edge types (dest = rc*512 +
    t*128 + r%128), so the GEMM for chunk rc starts as soon as window rc is
    scattered.
  - The node-feature table is split in two halves (A = each core's rows
    [0,3072), B = rows [3072,6272)) exchanged with two AllGathers per layer
    so the next layer's stream-A gathers start before stream B finishes.
  - Edge slots are padded only to the per-(window,stream) max across cores
    (~5% padding vs 28% for per-64-window tiles).
"""

import os
import sys
import numpy as np

sys.path.insert(0, "/opt/trn_rl_repo")

N = 50000
D = 128
NT = 4
NCORES = 8
RPC_REAL = 6250          # real rows per core
RPC = 6272               # padded rows per core (49*128)
NCHUNK = 49              # row chunks of 128 per core
NW = NCHUNK              # one 512-dest window per row chunk
WIN = 512
SPLITA = 3072            # per-core rows in table half A
SPLITB = RPC - SPLITA    # 3200
HA = NCORES * SPLITA     # 24576 rows in half A
HB = NCORES * SPLITB     # 25600 rows in half B
CT = 40                  # gather chunk size in 128-slot tiles
NEG_SLOPE = 0.01

_CACHE = {}


def _build_host_data(edge_index, edge_type, edge_attr):
    """Edge bookkeeping: per-core slot lists, int16 gather indices, S strips,
    and the static (uniform across cores) schedule."""
    row = edge_index[0].astype(np.int64)
    col = edge_index[1].astype(np.int64)
    et = edge_type.astype(np.int64)
    w = edge_attr.astype(np.float32)

    deg = np.bincount(col, minlength=N).astype(np.float32)

    owner = row // RPC_REAL
    r_loc = row - owner * RPC_REAL
    rc = r_loc // 128
    dest = rc * WIN + et * 128 + (r_loc % 128)   # within-core dest
    co = col // RPC_REAL
    c_loc = col - co * RPC_REAL
    s = (c_loc >= SPLITA).astype(np.int64)       # stream 0=A, 1=B
    tcol = np.where(s == 1, co * SPLITB + (c_loc - SPLITA), co * SPLITA + c_loc)

    # sort by (owner, stream, window, dest)
    order = np.lexsort((dest, rc, s, owner))
    dest_s, rc_s, s_s, own_s = dest[order], rc[order], s[order], owner[order]
    tcol_s, w_s = tcol[order], w[order]

    key = (own_s * 2 + s_s) * NW + rc_s
    cnt = np.bincount(key, minlength=NCORES * 2 * NW).reshape(NCORES, 2, NW)
    starts_flat = np.zeros(NCORES * 2 * NW + 1, dtype=np.int64)
    np.cumsum(cnt.reshape(-1), out=starts_flat[1:])

    u = cnt.max(axis=0)                          # [2, NW] padded bucket sizes
    ustart = np.zeros((2, NW), dtype=np.int64)   # slot offset of bucket
    U = np.zeros(2, dtype=np.int64)              # padded slots per stream
    ntile = np.zeros(2, dtype=np.int64)
    for st in range(2):
        ustart[st] = np.cumsum(u[st]) - u[st]
        U[st] = u[st].sum()
        ntile[st] = -(-U[st] // 128)

    # chunks: consecutive tiles
    chunks = []                                  # (stream, tile_lo, tile_hi)
    for st in range(2):
        t0 = 0
        while t0 < ntile[st]:
            t1 = min(t0 + CT, int(ntile[st]))
            chunks.append((st, t0, t1))
            t0 = t1
    # order chunks A0,A1,B0,A2,B1,... (B delayed one slot: tab-B's AllGather
    # finishes after tab-A's, so the first B gather would otherwise stall)
    ca = [c for c in chunks if c[0] == 0]
    cb = [c for c in chunks if c[0] == 1]
    chunks = ca[:2]
    for i in range(max(len(ca) - 2, len(cb))):
        if i < len(cb):
            chunks.append(cb[i])
        if i + 2 < len(ca):
            chunks.append(ca[i + 2])

    # per-(stream,tile) window strips: list of (rc, lo, width)
    # plus first/last bookkeeping per window for PSUM start/stop flags.
    # Slot -> core-local dest arrays for strip extents:
    dloc_s = dest_s - rc_s * WIN                 # 0..511 within window

    strips = [[] for _ in range(2)]              # per stream: per tile list
    for st in range(2):
        for t in range(int(ntile[st])):
            lo_sl, hi_sl = t * 128, t * 128 + 128
            # windows overlapping [lo_sl, hi_sl)
            tl = []
            wlo = int(np.searchsorted(ustart[st], lo_sl, side="right")) - 1
            whi = int(np.searchsorted(ustart[st], hi_sl, side="left"))
            for rcw in range(max(wlo, 0), min(whi, NW)):
                a = max(lo_sl, int(ustart[st][rcw]))
                b = min(hi_sl, int(ustart[st][rcw] + u[st][rcw]))
                if a >= b:
                    continue
                # strip extent = union over cores of dest range of real edges
                lo_d, hi_d = WIN, 0
                for k in range(NCORES):
                    base = starts_flat[(k * 2 + st) * NW + rcw]
                    nn = int(cnt[k, st, rcw])
                    e0 = a - int(ustart[st][rcw])
                    e1 = min(b - int(ustart[st][rcw]), nn)
                    if e0 >= e1:
                        continue
                    dd = dloc_s[base + e0: base + e1]
                    lo_d = min(lo_d, int(dd[0]))
                    hi_d = max(hi_d, int(dd[-1]) + 1)
                if lo_d >= hi_d:
                    lo_d, hi_d = 0, 1
                tl.append([rcw, lo_d, hi_d])
            strips[st].append(tl)

    # global emission order of scatter matmuls: by window rc, stream A strips
    # then B strips, tiles ascending.  first strip of each window is widened
    # to [0, WIN) (PSUM zero-init), flags start/stop assigned.
    win_strips = [[] for _ in range(NW)]         # (st, tile, lo, hi)
    for st in range(2):
        for t, tl in enumerate(strips[st]):
            for rcw, lo_d, hi_d in tl:
                win_strips[rcw].append([st, t, lo_d, hi_d])
    for rcw in range(NW):
        wl = sorted(win_strips[rcw], key=lambda x: (x[0], x[1]))
        wl[0][2], wl[0][3] = 0, WIN              # first strip full width
        win_strips[rcw] = wl

    # S column layout: strips stored per (stream, chunk) contiguously in
    # (tile, window) order; col offsets are uniform across cores.
    scol = {}                                    # (st, t, rcw) -> col offset
    chunk_scols = []                             # per chunk: (col_lo, col_hi)
    ccol = 0
    for (st, t0, t1) in chunks:
        c_lo = ccol
        for t in range(t0, t1):
            for rcw, lo_d, hi_d in _strips_of(strips, win_strips, st, t):
                scol[(st, t, rcw)] = ccol
                ccol += hi_d - lo_d
        chunk_scols.append((c_lo, ccol))
    stot = ccol

    # per-core data: idx arrays and S matrix
    per_core = []
    for k in range(NCORES):
        idxs = []
        for st in range(2):
            arr = np.zeros(int(ntile[st]) * 128, dtype=np.int16)
            for rcw in range(NW):
                base = starts_flat[(k * 2 + st) * NW + rcw]
                nn = int(cnt[k, st, rcw])
                off = int(ustart[st][rcw])
                arr[off:off + nn] = tcol_s[base:base + nn].astype(np.int16)
            idxs.append(arr)
        S = np.zeros((stot, 128), dtype=np.float32)
        for st in range(2):
            for t in range(int(ntile[st])):
                for rcw, lo_d, hi_d in _strips_of(strips, win_strips, st, t):
                    c0 = scol[(st, t, rcw)]
                    base = starts_flat[(k * 2 + st) * NW + rcw]
                    nn = int(cnt[k, st, rcw])
                    a = max(t * 128, int(ustart[st][rcw]))
                    b = min(t * 128 + 128, int(ustart[st][rcw]) + nn)
                    for sl in range(a, b):
                        e = base + (sl - int(ustart[st][rcw]))
                        d = int(dloc_s[e])
                        S[c0 + d - lo_d, sl - t * 128] = w_s[e]
        per_core.append({"idx": idxs, "S": S})

    meta = {
        "deg": deg, "u": u, "ustart": ustart, "U": U, "ntile": ntile,
        "chunks": chunks, "strips": strips, "win_strips": win_strips,
        "scol": scol, "chunk_scols": chunk_scols, "stot": stot,
    }
    return per_core, meta


def _strips_of(strips, win_strips, st, t):
    """Strips of tile t in stream st with final (lo, hi) after widening."""
    out = []
    for rcw, _lo, _hi in strips[st][t]:
        for st2, t2, lo2, hi2 in win_strips[rcw]:
            if st2 == st and t2 == t:
                out.append((rcw, lo2, hi2))
                break
    return out


def _build_program(meta):
    from concourse import bacc, mybir, tile

    dt = mybir.dt
    Afn = mybir.ActivationFunctionType
    Alu = mybir.AluOpType

    u = meta["u"]
    ustart = meta["ustart"]
    ntile = meta["ntile"]
    chunks = meta["chunks"]
    strips = meta["strips"]
    win_strips = meta["win_strips"]
    scol = meta["scol"]
    chunk_scols = meta["chunk_scols"]
    stot = meta["stot"]
    max_ccols = max(c1 - c0 for c0, c1 in chunk_scols)
    idx_cols = [int(ntile[0]) * 8, int(ntile[1]) * 8]   # 128 slots = 8 cols

    nc = bacc.Bacc("TRN2", target_bir_lowering=False, debug=False,
                   num_devices=NCORES, num_swdge_queues=2)

    x_in = nc.dram_tensor("x_in", [RPC, D], dt.float32, kind="ExternalInput").ap()
    deg_in = nc.dram_tensor("deg_in", [128, NCHUNK], dt.float32,
                            kind="ExternalInput").ap()
    idxa_in = nc.dram_tensor("idxa_in", [128, idx_cols[0]], dt.int16,
                             kind="ExternalInput").ap()
    idxb_in = nc.dram_tensor("idxb_in", [128, idx_cols[1]], dt.int16,
                             kind="ExternalInput").ap()
    s_in = nc.dram_tensor("s_in", [128, stot], dt.bfloat16,
                          kind="ExternalInput").ap()
    w_in = nc.dram_tensor("w_in", [3, NT, D, D], dt.float32,
                          kind="ExternalInput").ap()
    b_in = nc.dram_tensor("b_in", [3, 128, D], dt.float32,
                          kind="ExternalInput").ap()
    out_d = nc.dram_tensor("out_d", [RPC, D], dt.float32,
                           kind="ExternalOutput").ap()

    rg = [list(range(NCORES))]

    with tile.TileContext(nc) as tc:
        with tc.tile_pool(name="sb", bufs=1) as sb, \
             tc.tile_pool(name="gpool", bufs=6) as gpool, \
             tc.tile_pool(name="spool", bufs=3) as spool, \
             tc.tile_pool(name="mpool", bufs=3) as mpool, \
             tc.tile_pool(name="stage", bufs=4) as stage, \
             tc.tile_pool(name="psw", bufs=3, space="PSUM") as psw, \
             tc.tile_pool(name="psg", bufs=2, space="PSUM") as psg, \
             tc.tile_pool(name="dram", bufs=1, space="DRAM") as dram:

            # ---- persistent SBUF ----
            idx_sb = [sb.tile([128, idx_cols[0]], dt.int16, name="idxA"),
                      sb.tile([128, idx_cols[1]], dt.int16, name="idxB")]
            nc.sync.dma_start(idx_sb[0][:], idxa_in[:])
            nc.sync.dma_start(idx_sb[1][:], idxb_in[:])
            dis = sb.tile([128, NCHUNK], dt.float32)
            degt = sb.tile([128, NCHUNK], dt.float32)
            nc.sync.dma_start(degt[:], deg_in[:])
            # batched weight/bias loads: one DMA each (partition = row dim)
            W_sb = sb.tile([128, 3, NT, D], dt.bfloat16)
            wtmp = sb.tile([128, 3 * NT, D], dt.float32)
            nc.sync.dma_start(
                wtmp[:], w_in[:].rearrange("l t p d -> p (l t) d"))
            nc.vector.tensor_copy(
                W_sb[:].rearrange("p l t d -> p (l t d)"),
                wtmp[:].rearrange("p c d -> p (c d)"))
            B_sb = sb.tile([128, 3, D], dt.float32)
            nc.sync.dma_start(
                B_sb[:], b_in[:].rearrange("l p d -> p l d"))
            acc = sb.tile([128, NCHUNK, D], dt.float32)

            # dis = where(deg>0, 1/sqrt(deg), 0)
            deg1 = stage.tile([128, NCHUNK], dt.float32, tag="d1")
            nc.vector.tensor_scalar_max(deg1[:], degt[:], 1.0)
            inv = stage.tile([128, NCHUNK], dt.float32, tag="d2")
            nc.vector.reciprocal(inv[:], deg1[:])
            sq = stage.tile([128, NCHUNK], dt.float32, tag="d3")
            nc.scalar.activation(sq[:], inv[:], Afn.Sqrt)
            mask = stage.tile([128, NCHUNK], dt.float32, tag="d4")
            nc.vector.tensor_scalar(mask[:], degt[:], 0.0, None, op0=Alu.is_gt)
            nc.vector.tensor_tensor(dis[:], sq[:], mask[:], op=Alu.mult)
            # W/b are pre-scaled by 1/4 on the host (folds the final /4 into
            # each layer via lrelu homogeneity); table entries need dis*z =
            # 4*dis*z', so layer outputs are rescaled with dis4.
            dis4 = sb.tile([128, NCHUNK], dt.float32)
            nc.vector.tensor_scalar_mul(dis4[:], dis[:], 4.0)

            # ---- DRAM tables and per-layer input blocks ----
            tabA = [dram.tile([HA, D], dt.bfloat16, name=f"tabA{i}",
                              addr_space="Shared") for i in range(3)]
            tabB = [dram.tile([HB, D], dt.bfloat16, name=f"tabB{i}",
                              addr_space="Shared") for i in range(3)]
            blkA = [dram.tile([SPLITA, D], dt.bfloat16, name=f"blkA{i}")
                    for i in range(3)]
            blkB = [dram.tile([SPLITB, D], dt.bfloat16, name=f"blkB{i}")
                    for i in range(3)]

            # ---- prologue: acc = x/4 ; blk0 = dis * x ; AllGather halves ----
            # batched: one x load + one zt DMA per table half, staging reused
            NA = SPLITA // 128
            NB = NCHUNK - NA
            xh = sb.tile([128, NB, D], dt.float32)
            zth = sb.tile([128, NB, D], dt.bfloat16)
            nc.sync.dma_start(
                xh[:, :NA, :],
                x_in[:SPLITA, :].rearrange("(c p) d -> p c d", p=128))
            for rcc in range(NA):
                nc.scalar.activation(zth[:, rcc, :], xh[:, rcc, :], Afn.Copy,
                                     scale=dis[:, rcc:rcc + 1])
            nc.sync.dma_start(
                blkA[0][:].rearrange("(c p) d -> p c d", p=128),
                zth[:, :NA, :])
            nc.gpsimd.collective_compute(
                "AllGather", Alu.bypass, replica_groups=rg,
                ins=[blkA[0].opt()], outs=[tabA[0].opt()])
            nc.vector.tensor_scalar_mul(
                acc[:, :NA, :].rearrange("p c d -> p (c d)"),
                xh[:, :NA, :].rearrange("p c d -> p (c d)"), 0.25)
            nc.sync.dma_start(
                xh[:], x_in[SPLITA:, :].rearrange("(c p) d -> p c d", p=128))
            for rcc in range(NB):
                nc.scalar.activation(zth[:, rcc, :], xh[:, rcc, :], Afn.Copy,
                                     scale=dis[:, NA + rcc:NA + rcc + 1])
            nc.sync.dma_start(
                blkB[0][:].rearrange("(c p) d -> p c d", p=128), zth[:])
            nc.gpsimd.collective_compute(
                "AllGather", Alu.bypass, replica_groups=rg,
                ins=[blkB[0].opt()], outs=[tabB[0].opt()])
            nc.vector.tensor_scalar_mul(
                acc[:, NA:, :].rearrange("p c d -> p (c d)"),
                xh[:].rearrange("p c d -> p (c d)"), 0.25)

            # ---- layers ----
            gq = 0  # gather queue round-robin
            for l in range(3):
                table = [tabA[l], tabB[l]]
                # per-stream chunk tiles currently resident: dict chunk->tile
                gts = {}
                sts_ = {}
                chunk_of = [{}, {}]         # (stream, tile) -> chunk id
                for ci, (st, t0, t1) in enumerate(chunks):
                    for t in range(t0, t1):
                        chunk_of[st][t] = ci

                def emit_chunk(ci):
                    nonlocal gq
                    st, t0, t1 = chunks[ci]
                    nb = t1 - t0
                    gt = gpool.tile([128, CT, D], dt.bfloat16, tag="gt")
                    nc.gpsimd.dma_gather(
                        out_ap=gt[:, :nb, :],
                        in_ap=table[st][:],
                        idxs_ap=idx_sb[st][:, t0 * 8: t1 * 8],
                        num_idxs=nb * 128,
                        num_idxs_reg=nb * 128,
                        elem_size=D,
                        single_packet=False,
                        queue_num=gq,
                    )
                    gq ^= 1
                    c0, c1 = chunk_scols[ci]
                    ssb = spool.tile([128, max_ccols], dt.bfloat16, tag="ssb")
                    nc.sync.dma_start(ssb[:, :c1 - c0], s_in[:, c0:c1])
                    gts[ci] = gt
                    sts_[ci] = (ssb, c0)

                # which chunks must be emitted before window rcw's matmuls
                need_chunk = [[0, 0] for _ in range(NW)]
                for rcw in range(NW):
                    for st in range(2):
                        last_slot = int(ustart[st][rcw] + u[st][rcw]) - 1
                        need_chunk[rcw][st] = chunk_of[st][last_slot // 128]

                for rcw in range(NW):
                    for st in range(2):
                        # emit gather chunks up to the one containing rcw's end
                        nd = need_chunk[rcw][st]
                        for ci in range(len(chunks)):
                            if chunks[ci][0] == st and ci not in gts \
                                    and _chunk_ord(chunks, st, ci) <= \
                                    _chunk_ord(chunks, st, nd):
                                emit_chunk(ci)
                    # scatter matmuls for window rcw
                    pw = psw.tile([128, WIN], dt.float32, tag="pw")
                    wl = win_strips[rcw]
                    for i, (st, t, lo_d, hi_d) in enumerate(wl):
                        ci = chunk_of[st][t]
                        gt = gts[ci]
                        ssb, c0 = sts_[ci]
                        so = scol[(st, t, rcw)] - c0
                        nc.tensor.matmul(
                            out=pw[:, lo_d:hi_d],
                            lhsT=gt[:, t - chunks[ci][1], :],
                            rhs=ssb[:, so:so + hi_d - lo_d],
                            start=(i == 0),
                            stop=(i == len(wl) - 1),
                        )
                    # evict window -> msg bf16
                    msg = mpool.tile([128, WIN], dt.bfloat16, tag="msg")
                    nc.scalar.activation(msg[:], pw[:], Afn.Copy)
                    # GEMM for row chunk rcw
                    pg = psg.tile([128, D], dt.float32, tag="pg")
                    for t in range(NT):
                        nc.tensor.matmul(
                            out=pg[:],
                            lhsT=msg[:, t * 128:(t + 1) * 128],
                            rhs=W_sb[:, l, t, :],
                            start=(t == 0),
                            stop=(t == NT - 1),
                        )
                    tz = stage.tile([128, D], dt.float32, tag="tz")
                    nc.scalar.activation(tz[:], pg[:], Afn.Copy,
                                         scale=dis[:, rcw:rcw + 1])
                    nc.vector.tensor_tensor(tz[:], tz[:], B_sb[:, l, :],
                                            op=Alu.add)
                    z = stage.tile([128, D], dt.float32, tag="z")
                    nc.scalar.activation(z[:], tz[:], Afn.Lrelu,
                                         alpha=NEG_SLOPE)
                    if l < 2:
                        nc.vector.tensor_tensor(acc[:, rcw, :], acc[:, rcw, :],
                                                z[:], op=Alu.add)
                        zt = stage.tile([128, D], dt.bfloat16, tag="zt2")
                        nc.scalar.activation(zt[:], z[:], Afn.Copy,
                                             scale=dis4[:, rcw:rcw + 1])
                        if rcw < SPLITA // 128:
                            nc.sync.dma_start(
                                blkA[l + 1][rcw * 128:(rcw + 1) * 128, :], zt[:])
                        else:
                            r2 = rcw - SPLITA // 128
                            nc.sync.dma_start(
                                blkB[l + 1][r2 * 128:(r2 + 1) * 128, :], zt[:])
                        if rcw == 24:
                            nc.gpsimd.collective_compute(
                                "AllGather", Alu.bypass, replica_groups=rg,
                                ins=[blkA[l + 1].opt()],
                                outs=[tabA[l + 1].opt()])
                    else:
                        # final layer: out = acc + z' (the /4 is pre-folded
                        # into W/b, and acc accumulates 0.25-scaled terms)
                        oc = stage.tile([128, D], dt.float32, tag="oc")
                        nc.vector.tensor_tensor(oc[:], acc[:, rcw, :], z[:],
                                                op=Alu.add)
                        nc.sync.dma_start(out_d[rcw * 128:(rcw + 1) * 128, :],
                                          oc[:])
                if l < 2:
                    nc.gpsimd.collective_compute(
                        "AllGather", Alu.bypass, replica_groups=rg,
                        ins=[blkB[l + 1].opt()], outs=[tabB[l + 1].opt()])

    nc.compile()
    return nc


def _chunk_ord(chunks, st, ci):
    """Ordinal of chunk ci within its stream."""
    n = 0
    for j in range(ci):
        if chunks[j][0] == st:
            n += 1
    return n


def kernel(x, edge_index, edge_type, edge_attr, W1, b1, W2, b2, W3, b3):
    from concourse import bass_utils
    import ml_dtypes

    key = "prog"
    per_core, meta = _build_host_data(edge_index, edge_type, edge_attr)
    if key not in _CACHE:
        _CACHE[key] = _build_program(meta)
    nc = _CACHE[key]

    x = np.asarray(x, dtype=np.float32)
    deg = meta["deg"]
    # final /4 folded into each layer: lrelu(0.25*(pre)) = 0.25*lrelu(pre)
    Ws = 0.25 * np.stack([np.asarray(W1), np.asarray(W2),
                          np.asarray(W3)]).astype(np.float32)
    Bs = 0.25 * np.stack([np.tile(np.asarray(b)[None, :], (128, 1))
                          for b in (b1, b2, b3)]).astype(np.float32)

    in_maps = []
    for k in range(NCORES):
        xblk = np.zeros((RPC, D), dtype=np.float32)
        xblk[:RPC_REAL] = x[k * RPC_REAL:(k + 1) * RPC_REAL]
        degw = np.zeros((128, NCHUNK), dtype=np.float32)
        dblk = np.zeros(RPC, dtype=np.float32)
        dblk[:RPC_REAL] = deg[k * RPC_REAL:(k + 1) * RPC_REAL]
        degw[:, :] = dblk.reshape(NCHUNK, 128).T
        idxs = per_core[k]["idx"]
        idx_maps = []
        for st in range(2):
            arr = idxs[st].reshape(-1, 16)        # [cols, 16]
            idx_maps.append(np.tile(arr.T, (8, 1)))  # [128, cols]
        S_bf = per_core[k]["S"].T.astype(ml_dtypes.bfloat16)  # [128, stot]
        in_maps.append({
            "x_in": xblk,
            "deg_in": degw,
            "idxa_in": idx_maps[0],
            "idxb_in": idx_maps[1],
            "s_in": S_bf,
            "w_in": Ws,
            "b_in": Bs,
        })

    trace = bool(int(os.environ.get("KTRACE", "0")))
    res = bass_utils.run_bass_kernel_spmd(nc, in_maps, core_ids=list(range(NCORES)),
                                          trace=trace)
    global _LAST_EXEC_NS, _LAST_RES
    _LAST_EXEC_NS = res.exec_time_ns
    _LAST_RES = res
    out = np.zeros((N, D), dtype=np.float32)
    for k in range(NCORES):
        out[k * RPC_REAL:(k + 1) * RPC_REAL] = res.results[k]["out_d"][:RPC_REAL]
    return out
